# revision 1
# baseline (speedup 1.0000x reference)
"""GCN (5-layer PyG GCNConv + BatchNorm eval + ReLU) on 8 Trainium2 NeuronCores.

Sharding: nodes are dst-sharded across the 8 cores (12544 padded rows each);
edges follow their destination. Per layer, each core computes h = act @ W'
for its own nodes (BN folded into W'/b' on the host), scales by dinv, then an
AllGather makes the scaled activations visible to every core; aggregation
runs per 128-node destination tile via dma_gather (512B rows, 4 SWDGE queues
in parallel) feeding one-hot selection matmuls that accumulate in PSUM.
Edge order, gather indices (int16, relative to 32768-row chunks) and one-hot
selection values are all precomputed on the host from edge_index.
"""
import os
import numpy as np

N = 100000
E = 1600000
IN = 128
H = 128
C = 2
EPS = 1e-5
NC = 8
SR = 12500            # real nodes per core
P = 128
TP = 98               # dst tiles per core
SH = TP * P           # padded nodes per core = 12544
NF = SH * NC          # padded total = 100352
CH = 32768            # gather-source chunk rows (int16-addressable)
NCHUNK = 4
CH_BASE = [0, CH, 2 * CH, 3 * CH]
CH_SIZE = [CH, CH, CH, NF - 3 * CH]
DIMS = [(IN, H), (H, H), (H, H), (H, H // 2), (H // 2, C)]
AGG_D = [128, 128, 128, 64, 64]   # gathered row width per layer's aggregation

_cache = {}

# ---------------------------------------------------------------------------
# Tile patch: walrus in this container rejects TPB_CTRL/extended instructions
# with >1 sync wait. Split waits across single-wait NOPs.
# ---------------------------------------------------------------------------


def _apply_tile_patch():
    if _cache.get("patched"):
        return
    _cache["patched"] = True
    import concourse.tile as tile_mod
    import concourse.mybir as mybir
    from concourse.vector_clock import ScopedClock

    MAXW = 1

    def _patched_drain_and_barrier(self, tick_clock, wait_clock):
        nc = self.nc
        probe = nc.sync.nop(nofuse=True)
        wait_clock.add_sem_waits(probe.ins, ScopedClock({None: tick_clock.global_clock}))
        si = probe.ins.sync_info
        if si is not None and si.on_wait and len(si.on_wait) > MAXW:
            waits = list(si.on_wait)
            si.on_wait = waits[:MAXW]
            for k in range(MAXW, len(waits), MAXW):
                extra = nc.sync.nop(nofuse=True)
                esi = extra.ins.sync_info
                if esi is None:
                    extra.ins.sync_info = mybir.SyncInfo(
                        on_wait=waits[k:k + MAXW], on_update=[]
                    )
                else:
                    esi.on_wait = waits[k:k + MAXW]
        nc.sync.drain()
        nc.all_engine_barrier()
        assert self.sems is not None
        popped = nc._tile_sem_poison_stack.pop()
        assert popped is self._sem_poison
        nc.clear_and_free_semaphores(list(self.sems.allocated().values()))
        nc.all_engine_barrier()

    tile_mod.TileContext._drain_and_barrier = _patched_drain_and_barrier

    _orig_commit = tile_mod.TileContext._commit_instruction

    def _patched_commit_instruction(self, inst, lazy_reg_writes=True):
        si = getattr(inst, "sync_info", None)
        if (
            si is not None
            and si.on_wait
            and len(si.on_wait) > MAXW
            and inst.engine != mybir.EngineType.Unassigned
        ):
            waits = list(si.on_wait)
            si.on_wait = waits[:MAXW]
            eng = self.nc.engines[inst.engine]
            for k in range(MAXW, len(waits), MAXW):
                extra = eng.nop(nofuse=True)
                esi = extra.ins.sync_info
                chunk = waits[k:k + MAXW]
                if esi is None:
                    extra.ins.sync_info = mybir.SyncInfo(on_wait=chunk, on_update=[])
                else:
                    esi.on_wait = chunk
        return _orig_commit(self, inst, lazy_reg_writes)

    tile_mod.TileContext._commit_instruction = _patched_commit_instruction


# ---------------------------------------------------------------------------
# SPMD runner: compile once via bass2jax/PJRT, keep the jitted fn for reuse.
# ---------------------------------------------------------------------------


class _SpmdRunner:
    def __init__(self, nc, n_cores=8):
        import jax
        from jax.sharding import Mesh, PartitionSpec, NamedSharding
        from jax.experimental.shard_map import shard_map
        import concourse.mybir as mybir
        from concourse.bass2jax import (
            _bass_exec_p,
            install_neuronx_cc_hook,
            partition_id_tensor,
        )
        from concourse.library_overlay import lower_extended_insts

        lower_extended_insts(nc)
        install_neuronx_cc_hook()
        self.jax = jax
        self.n_cores = n_cores
        partition_name = nc.partition_id_tensor.name if nc.partition_id_tensor else None
        in_names, out_names, out_avals, zero_outs = [], [], [], []
        for alloc in nc.m.functions[0].allocations:
            if not isinstance(alloc, mybir.MemoryLocationSet):
                continue
            name = alloc.memorylocations[0].name
            if alloc.kind == "ExternalInput":
                if name != partition_name:
                    in_names.append(name)
            elif alloc.kind == "ExternalOutput":
                out_names.append(name)
                shape = tuple(alloc.tensor_shape)
                dtype = mybir.dt.np(alloc.dtype)
                out_avals.append(jax.core.ShapedArray(shape, dtype))
                zero_outs.append(np.zeros(shape, dtype))
        self.in_names = list(in_names)
        self.out_names = out_names
        self.out_avals = out_avals
        self.zero_outs = zero_outs
        n_params = len(in_names)
        n_outs = len(out_avals)
        all_in_names = list(in_names) + list(out_names)
        if partition_name is not None:
            all_in_names.append(partition_name)

        def _body(*args):
            operands = list(args)
            if partition_name is not None:
                operands.append(partition_id_tensor())
            outs = _bass_exec_p.bind(
                *operands,
                out_avals=tuple(out_avals),
                in_names=tuple(all_in_names),
                out_names=tuple(out_names),
                lowering_input_output_aliases=(),
                sim_require_finite=True,
                sim_require_nnan=True,
                nc=nc,
            )
            return tuple(outs)

        devices = jax.devices()[:n_cores]
        self.mesh = Mesh(np.asarray(devices), ("core",))
        in_specs = (PartitionSpec("core"),) * (n_params + n_outs)
        out_specs = (PartitionSpec("core"),) * n_outs
        self.sharding = NamedSharding(self.mesh, PartitionSpec("core"))
        self.fn = jax.jit(
            shard_map(
                _body, mesh=self.mesh, in_specs=in_specs, out_specs=out_specs,
                check_rep=False,
            ),
            keep_unused=True,
        )
        self.n_params = n_params

    def put_inputs(self, in_maps):
        jax = self.jax
        per_core = [[np.asarray(m[name]) for name in self.in_names] for m in in_maps]
        concat_in = [
            np.concatenate([per_core[c][i] for c in range(self.n_cores)], axis=0)
            for i in range(self.n_params)
        ]
        self.dev_in = [jax.device_put(a, self.sharding) for a in concat_in]
        self.dev_zeros = [
            jax.device_put(
                np.zeros((self.n_cores * z.shape[0], *z.shape[1:]), z.dtype),
                self.sharding,
            )
            for z in self.zero_outs
        ]
        jax.block_until_ready(self.dev_in)

    def run(self):
        outs = self.fn(*self.dev_in, *self.dev_zeros)
        self.jax.block_until_ready(outs)
        return outs

    def results(self, outs):
        res = []
        for c in range(self.n_cores):
            res.append(
                {
                    name: np.asarray(outs[i]).reshape(
                        self.n_cores, *self.out_avals[i].shape
                    )[c]
                    for i, name in enumerate(self.out_names)
                }
            )
        return res

    def time_runs(self, n=6):
        import time
        ts = []
        for _ in range(n):
            t0 = time.perf_counter()
            self.run()
            ts.append(time.perf_counter() - t0)
        return ts


# ---------------------------------------------------------------------------
# Host-side graph partitioning
# ---------------------------------------------------------------------------


def _host_prep(edge_index):
    src = np.asarray(edge_index[0], dtype=np.int64)
    dst = np.asarray(edge_index[1], dtype=np.int64)
    deg = np.bincount(dst, minlength=N).astype(np.float32) + 1.0
    dinv = (1.0 / np.sqrt(deg)).astype(np.float32)

    core = dst // SR
    dl = dst - core * SR
    tile = dl // P
    dslot = dl % P
    # src ids remapped to padded positions so gathers hit the padded table
    score = src // SR
    psrc = score * SH + (src - score * SR)
    chunk = psrc // CH
    crel = psrc - chunk * CH

    gid = ((core * TP + tile) * NCHUNK + chunk).astype(np.int64)
    order = np.lexsort((psrc, gid))
    gid_s = gid[order]
    crel_s = crel[order]
    dslot_s = dslot[order]

    ngroups = NC * TP * NCHUNK
    cnt = np.bincount(gid_s, minlength=ngroups)
    cnt4 = cnt.reshape(NC, TP, NCHUNK)
    bcap = [max(1, int(np.ceil(cnt4[:, :, ch].max() / P))) for ch in range(NCHUNK)]
    TB = sum(bcap)
    blkoff = np.cumsum([0] + bcap)[:NCHUNK]

    gstart = np.zeros(ngroups + 1, np.int64)
    np.cumsum(cnt, out=gstart[1:])
    rank = np.arange(E) - gstart[gid_s]
    ch_s = gid_s % NCHUNK
    t_s = (gid_s // NCHUNK) % TP
    c_s = gid_s // (NCHUNK * TP)
    pos = t_s * (TB * P) + blkoff[ch_s] * P + rank

    idx_tab = np.zeros((NC, TP * TB * P), np.int16)
    dsel_tab = np.full((NC, TP * TB * P), -1.0, np.float32)
    idx_tab[c_s, pos] = crel_s.astype(np.int16)
    dsel_tab[c_s, pos] = dslot_s.astype(np.float32)

    # wrapped idx layout per tile [128, TB*8] int16; element i of a
    # (tile,chunk) segment sits at [16k + i%16, i//16] for k in 0..7 (the 8
    # Q7 GPSIMD cores each read their own 16-partition window)
    idx4 = idx_tab.reshape(NC, TP, TB * P)
    idx_w = np.zeros((NC, TP, P, TB * 8), np.int16)
    col = 0
    for ch in range(NCHUNK):
        n = bcap[ch] * P
        seg = idx4[:, :, blkoff[ch] * P: blkoff[ch] * P + n]
        w16 = seg.reshape(NC, TP, n // 16, 16).transpose(0, 1, 3, 2)
        idx_w[:, :, :, col: col + n // 16] = np.tile(w16, (1, 1, 8, 1))
        col += n // 16
    dsel_w = dsel_tab.reshape(NC, TP, TB, P).transpose(0, 1, 3, 2).copy()

    return dinv, idx_w, dsel_w, bcap, TB, blkoff


def _fold_weights(inputs):
    Ws, Bs = [], []
    for i in range(1, 6):
        W = np.asarray(inputs[f"W{i}"], np.float32)
        b = np.asarray(inputs[f"b{i}"], np.float32)
        if i <= 4:
            g = np.asarray(inputs[f"g{i}"], np.float32)
            be = np.asarray(inputs[f"be{i}"], np.float32)
            rm = np.asarray(inputs[f"rm{i}"], np.float32)
            rv = np.asarray(inputs[f"rv{i}"], np.float32)
            s = g / np.sqrt(rv + EPS)
            W = W * s[None, :]
            b = b * s + be - rm * s
        Ws.append(np.ascontiguousarray(W, dtype=np.float32))
        Bs.append(np.tile(b[None, :].astype(np.float32), (P, 1)))
    return Ws, Bs


# ---------------------------------------------------------------------------
# Device program
# ---------------------------------------------------------------------------


def _build_nc(bcap, TB, blkoff):
    TPB = int(os.environ.get("GCN_TILES", TP))
    NLAY = int(os.environ.get("GCN_LAYERS", 5))
    import concourse.bass as bass
    import concourse.mybir as mybir
    from concourse.tile import TileContext
    from concourse import library_config

    _apply_tile_patch()

    f32 = mybir.dt.float32
    nc = bass.Bass("TRN2", target_bir_lowering=False, debug=False, num_swdge_queues=4)

    xT_in = nc.declare_dram_parameter("xT", [IN, SH], f32, isOutput=False)
    dinv_in = nc.declare_dram_parameter("dinv", [P, TP], f32, isOutput=False)
    idx_in = nc.declare_dram_parameter("idx", [TP, P, TB * 8], mybir.dt.int16, isOutput=False)
    dsel_in = nc.declare_dram_parameter("dsel", [TP, P, TB], f32, isOutput=False)
    W_in = [nc.declare_dram_parameter(f"W{i+1}", list(DIMS[i]), f32, isOutput=False) for i in range(5)]
    B_in = [nc.declare_dram_parameter(f"B{i+1}", [P, DIMS[i][1]], f32, isOutput=False) for i in range(5)]
    iota_in = nc.declare_dram_parameter("iota", [P, P], f32, isOutput=False)
    ident_in = nc.declare_dram_parameter("ident", [P, P], f32, isOutput=False)
    y_out = nc.declare_dram_parameter("y", [SH, C], f32, isOutput=True)

    in_b = [nc.dram_tensor(f"in_b{l}", [SH, AGG_D[l]], f32) for l in range(5)]
    hs_full = [
        nc.dram_tensor(f"hs_full{l}", [NF, AGG_D[l]], f32, addr_space="Shared")
        for l in range(5)
    ]

    with TileContext(nc) as tc:
        with (
            tc.tile_pool(name="const", bufs=1) as cpool,
            tc.tile_pool(name="act", bufs=1) as apool,
            tc.tile_pool(name="gath", bufs=4) as gpool,
            tc.tile_pool(name="idxp", bufs=8) as ipool,
            tc.tile_pool(name="sp", bufs=6) as spool,
            tc.tile_pool(name="work", bufs=4) as wpool,
            tc.tile_pool(name="ps_h", bufs=2, space="PSUM") as ps_h,
            tc.tile_pool(name="ps_a", bufs=3, space="PSUM") as ps_a,
            tc.tile_pool(name="ps_t", bufs=2, space="PSUM") as ps_t,
            tc.tile_pool(name="ps_o", bufs=1, space="PSUM") as ps_o,
        ):
            nc.gpsimd.load_library(library_config.mlp)
            nid_regs = []
            for ch in range(NCHUNK):
                r = nc.alloc_register(mybir.EngineType.Pool, f"nidx{ch}")
                nc.gpsimd.reg_mov(r, bcap[ch] * P)
                nid_regs.append(r)

            Wt, Bt = [], []
            for l in range(5):
                w = cpool.tile(list(DIMS[l]), f32, name=f"Wt{l}")
                nc.sync.dma_start(out=w[:], in_=W_in[l][:])
                Wt.append(w)
                b = cpool.tile([P, DIMS[l][1]], f32, name=f"Bt{l}")
                nc.sync.dma_start(out=b[:], in_=B_in[l][:])
                Bt.append(b)
            iota_t = cpool.tile([P, P], f32)
            nc.sync.dma_start(out=iota_t[:], in_=iota_in[:])
            ident_t = cpool.tile([P, P], f32)
            nc.sync.dma_start(out=ident_t[:], in_=ident_in[:])
            dinv_t = cpool.tile([P, TP], f32)
            nc.sync.dma_start(out=dinv_t[:], in_=dinv_in[:])
            actT = apool.tile([P, SH], f32)
            nc.sync.dma_start(out=actT[:IN, :], in_=xT_in[:])

            def gather_tile(l, t, gt):
                for ch in range(NCHUNK):
                    it = ipool.tile([P, bcap[ch] * 8], mybir.dt.int16, tag=f"idx{ch}")
                    nc.scalar.dma_start(
                        out=it[:],
                        in_=idx_in.ap()[t, :, blkoff[ch] * 8: (blkoff[ch] + bcap[ch]) * 8],
                    )
                    nc.gpsimd.dma_gather(
                        out_ap=gt[:, blkoff[ch]: blkoff[ch] + bcap[ch], :],
                        in_ap=hs_full[l].ap()[CH_BASE[ch]: CH_BASE[ch] + CH_SIZE[ch], :],
                        idxs_ap=it[:],
                        num_idxs=bcap[ch] * P,
                        num_idxs_reg=nid_regs[ch],
                        elem_size=AGG_D[l],
                        single_packet=False,
                        queue_num=(ch + t) % 4,
                    )

            for l in range(NLAY):
                I, O = DIMS[l]
                D = AGG_D[l]
                if l < 4:
                    for t in range(TPB):
                        ps = ps_h.tile([P, O], f32, tag="ps_h")
                        nc.tensor.matmul(
                            out=ps[:], lhsT=actT[:I, t * P:(t + 1) * P], rhs=Wt[l][:],
                            start=True, stop=True,
                        )
                        hs_t = wpool.tile([P, O], f32, tag="hs")
                        nc.vector.tensor_scalar_mul(
                            out=hs_t[:], in0=ps[:], scalar1=dinv_t[:, t:t + 1]
                        )
                        nc.sync.dma_start(out=in_b[l].ap()[t * P:(t + 1) * P, :], in_=hs_t[:])
                nc.gpsimd.collective_compute(
                    "AllGather",
                    mybir.AluOpType.bypass,
                    ins=[in_b[l][:]],
                    outs=[hs_full[l][:]],
                    replica_groups=[list(range(NC))],
                )
                for t in range(TPB):
                    gt = gpool.tile([P, TB, D], f32, tag="g")
                    gather_tile(l, t, gt)
                    dst_t = spool.tile([P, TB], f32, tag="dsel")
                    nc.scalar.dma_start(out=dst_t[:], in_=dsel_in.ap()[t])
                    pa = ps_a.tile([P, D], f32, tag="pa")
                    for b in range(TB):
                        S = spool.tile([P, P], f32, tag="S")
                        nc.vector.tensor_scalar(
                            out=S[:], in0=iota_t[:], scalar1=dst_t[:, b:b + 1],
                            scalar2=None, op0=mybir.AluOpType.is_equal,
                        )
                        nc.tensor.matmul(
                            out=pa[:], lhsT=S[:], rhs=gt[:, b, :],
                            start=(b == 0), stop=(b == TB - 1),
                        )
                    hso = wpool.tile([P, D], f32, tag="hso")
                    nc.sync.dma_start(out=hso[:], in_=in_b[l].ap()[t * P:(t + 1) * P, :])
                    u = wpool.tile([P, D], f32, tag="u")
                    nc.vector.tensor_add(out=u[:], in0=pa[:], in1=hso[:])
                    v = wpool.tile([P, D], f32, tag="v")
                    nc.vector.tensor_scalar_mul(out=v[:], in0=u[:], scalar1=dinv_t[:, t:t + 1])
                    if l < 4:
                        w_ = wpool.tile([P, D], f32, tag="w")
                        nc.vector.tensor_add(out=w_[:], in0=v[:], in1=Bt[l][:])
                        act_t = wpool.tile([P, D], f32, tag="actn")
                        nc.scalar.activation(
                            out=act_t[:], in_=w_[:],
                            func=mybir.ActivationFunctionType.Relu,
                        )
                        if l == 3:
                            hs5 = wpool.tile([P, D], f32, tag="hs5")
                            nc.vector.tensor_scalar_mul(
                                out=hs5[:], in0=act_t[:], scalar1=dinv_t[:, t:t + 1]
                            )
                            nc.sync.dma_start(
                                out=in_b[4].ap()[t * P:(t + 1) * P, :], in_=hs5[:]
                            )
                        else:
                            pt = ps_t.tile([P, P], f32, tag="pt")
                            nc.tensor.transpose(out=pt[:D, :], in_=act_t[:], identity=ident_t[:])
                            nc.vector.tensor_copy(out=actT[:D, t * P:(t + 1) * P], in_=pt[:D, :])
                    else:
                        pt = ps_t.tile([P, P], f32, tag="pt")
                        nc.tensor.transpose(out=pt[:D, :], in_=v[:], identity=ident_t[:])
                        vT = wpool.tile([P, P], f32, tag="vT")
                        nc.vector.tensor_copy(out=vT[:D, :], in_=pt[:D, :])
                        po = ps_o.tile([P, C], f32, tag="po")
                        nc.tensor.matmul(out=po[:], lhsT=vT[:D, :], rhs=Wt[4][:], start=True, stop=True)
                        yt = wpool.tile([P, C], f32, tag="yt")
                        nc.vector.tensor_add(out=yt[:], in0=po[:], in1=Bt[4][:])
                        nc.sync.dma_start(out=y_out.ap()[t * P:(t + 1) * P, :], in_=yt[:])
    return nc


def kernel(**inputs):
    edge_index = np.asarray(inputs["edge_index"])
    key = edge_index.tobytes()[:64]
    if "prep" not in _cache or _cache.get("key") != key:
        _cache["key"] = key
        _cache["prep"] = _host_prep(edge_index)
        _cache.pop("runner", None)
    dinv, idx_w, dsel_w, bcap, TB, blkoff = _cache["prep"]
    Ws, Bs = _fold_weights(inputs)

    x = np.asarray(inputs["x"], np.float32)
    xpad = np.zeros((NC, SH, IN), np.float32)
    xpad[:, :SR] = x.reshape(NC, SR, IN)
    xpad = xpad.reshape(NF, IN)
    dinvpad = np.ones((NC, SH), np.float32)
    dinvpad[:, :SR] = dinv.reshape(NC, SR)
    dinvpad = dinvpad.reshape(NF)

    iota = np.tile(np.arange(P, dtype=np.float32)[None, :], (P, 1))

    if "runner" not in _cache:
        nc = _build_nc(bcap, TB, blkoff)
        _cache["runner"] = _SpmdRunner(nc, NC)
    r = _cache["runner"]

    in_maps = []
    for c in range(NC):
        m = {
            "xT": np.ascontiguousarray(xpad[c * SH:(c + 1) * SH].T),
            "dinv": np.ascontiguousarray(dinvpad[c * SH:(c + 1) * SH].reshape(TP, P).T),
            "idx": idx_w[c],
            "dsel": dsel_w[c],
            "iota": iota,
            "ident": np.eye(P, dtype=np.float32),
        }
        for i in range(5):
            m[f"W{i+1}"] = Ws[i]
            m[f"B{i+1}"] = Bs[i]
        in_maps.append(m)

    r.put_inputs(in_maps)
    outs = r.run()
    res = r.results(outs)
    y = np.concatenate([res[c]["y"][:SR] for c in range(NC)], axis=0)[:N]
    return np.ascontiguousarray(y, dtype=np.float32)



# revision 2
# speedup vs baseline: 12.5993x; 12.5993x over previous
"""GCN (5-layer PyG GCNConv + BatchNorm eval + ReLU) on 8 Trainium2 NeuronCores, v3.

Key ideas vs v2:
- dst-degree scaling deferred through ReLU (dinv>0): store a_l = relu(pa +
  b*sqrtdeg) per tile; apply dinv^2 as the exact per-node phase-A scalar. The
  bias enters the PSUM accumulation as a rank-1 bf16 matmul (b_row x sqrtdeg
  row), so no per-tile broadcast/multiply chain exists at all. The final
  layer's dinv scaling and bias move to the host.
- Dense per-(group,chunk) gather segments with per-core exact descriptor
  counts (reg_load from an SBUF counts table) and trailing -1 padding: only
  real edges generate DMA descriptors. One dma_gather per (4-tile group x
  chunk). Per-tile one-hot windows are the cross-core union of block ranges,
  masked per-core via dsel = -1.
- One batched is_equal per tile builds all one-hot S blocks (free-dim
  0-stride broadcast of iota and dsel), bf16 at 2x DVE rate.
"""
import os
import numpy as np

N = 100000
E = 1600000
IN = 128
H = 128
C = 2
EPS = 1e-5
NC = 8
SR = 12500            # real nodes per core
P = 128
TP = 98               # dst tiles per core
SH = TP * P           # padded nodes per core = 12544
NF = SH * NC          # padded total = 100352
CH = 32768            # gather-source chunk rows (int16-addressable)
NCHUNK = 4
CH_BASE = [0, CH, 2 * CH, 3 * CH]
CH_SIZE = [CH, CH, CH, NF - 3 * CH]
GK = 4                # dst tiles per merged-gather group
NG = (TP + GK - 1) // GK
GROUP_TILES = [list(range(g * GK, min((g + 1) * GK, TP))) for g in range(NG)]

_cache = {}

# ---------------------------------------------------------------------------
# Tile patch: walrus in this container rejects TPB_CTRL/extended instructions
# with >1 sync wait. Split waits across single-wait NOPs.
# ---------------------------------------------------------------------------


def _apply_tile_patch():
    if _cache.get("patched"):
        return
    _cache["patched"] = True
    import concourse.tile as tile_mod
    import concourse.mybir as mybir
    from concourse.vector_clock import ScopedClock

    MAXW = 1

    def _patched_drain_and_barrier(self, tick_clock, wait_clock):
        nc = self.nc
        probe = nc.sync.nop(nofuse=True)
        wait_clock.add_sem_waits(probe.ins, ScopedClock({None: tick_clock.global_clock}))
        si = probe.ins.sync_info
        if si is not None and si.on_wait and len(si.on_wait) > MAXW:
            waits = list(si.on_wait)
            si.on_wait = waits[:MAXW]
            for k in range(MAXW, len(waits), MAXW):
                extra = nc.sync.nop(nofuse=True)
                esi = extra.ins.sync_info
                if esi is None:
                    extra.ins.sync_info = mybir.SyncInfo(
                        on_wait=waits[k:k + MAXW], on_update=[]
                    )
                else:
                    esi.on_wait = waits[k:k + MAXW]
        nc.sync.drain()
        nc.all_engine_barrier()
        assert self.sems is not None
        popped = nc._tile_sem_poison_stack.pop()
        assert popped is self._sem_poison
        nc.clear_and_free_semaphores(list(self.sems.allocated().values()))
        nc.all_engine_barrier()

    tile_mod.TileContext._drain_and_barrier = _patched_drain_and_barrier

    _orig_commit = tile_mod.TileContext._commit_instruction

    def _patched_commit_instruction(self, inst, lazy_reg_writes=True):
        si = getattr(inst, "sync_info", None)
        if (
            si is not None
            and si.on_wait
            and len(si.on_wait) > MAXW
            and inst.engine != mybir.EngineType.Unassigned
        ):
            waits = list(si.on_wait)
            si.on_wait = waits[:MAXW]
            eng = self.nc.engines[inst.engine]
            for k in range(MAXW, len(waits), MAXW):
                extra = eng.nop(nofuse=True)
                esi = extra.ins.sync_info
                chunk = waits[k:k + MAXW]
                if esi is None:
                    extra.ins.sync_info = mybir.SyncInfo(on_wait=chunk, on_update=[])
                else:
                    esi.on_wait = chunk
        return _orig_commit(self, inst, lazy_reg_writes)

    tile_mod.TileContext._commit_instruction = _patched_commit_instruction


# ---------------------------------------------------------------------------
# SPMD runner
# ---------------------------------------------------------------------------


class _SpmdRunner:
    def __init__(self, nc, n_cores=8):
        import jax
        from jax.sharding import Mesh, PartitionSpec, NamedSharding
        from jax.experimental.shard_map import shard_map
        import concourse.mybir as mybir
        from concourse.bass2jax import (
            _bass_exec_p,
            install_neuronx_cc_hook,
            partition_id_tensor,
        )
        from concourse.library_overlay import lower_extended_insts

        lower_extended_insts(nc)
        install_neuronx_cc_hook()
        self.jax = jax
        self.n_cores = n_cores
        partition_name = nc.partition_id_tensor.name if nc.partition_id_tensor else None
        in_names, out_names, out_avals, zero_outs = [], [], [], []
        for alloc in nc.m.functions[0].allocations:
            if not isinstance(alloc, mybir.MemoryLocationSet):
                continue
            name = alloc.memorylocations[0].name
            if alloc.kind == "ExternalInput":
                if name != partition_name:
                    in_names.append(name)
            elif alloc.kind == "ExternalOutput":
                out_names.append(name)
                shape = tuple(alloc.tensor_shape)
                dtype = mybir.dt.np(alloc.dtype)
                out_avals.append(jax.core.ShapedArray(shape, dtype))
                zero_outs.append(np.zeros(shape, dtype))
        self.in_names = list(in_names)
        self.out_names = out_names
        self.out_avals = out_avals
        self.zero_outs = zero_outs
        n_params = len(in_names)
        n_outs = len(out_avals)
        all_in_names = list(in_names) + list(out_names)
        if partition_name is not None:
            all_in_names.append(partition_name)

        def _body(*args):
            operands = list(args)
            if partition_name is not None:
                operands.append(partition_id_tensor())
            outs = _bass_exec_p.bind(
                *operands,
                out_avals=tuple(out_avals),
                in_names=tuple(all_in_names),
                out_names=tuple(out_names),
                lowering_input_output_aliases=(),
                sim_require_finite=True,
                sim_require_nnan=True,
                nc=nc,
            )
            return tuple(outs)

        devices = jax.devices()[:n_cores]
        self.mesh = Mesh(np.asarray(devices), ("core",))
        in_specs = (PartitionSpec("core"),) * (n_params + n_outs)
        out_specs = (PartitionSpec("core"),) * n_outs
        self.sharding = NamedSharding(self.mesh, PartitionSpec("core"))
        self.fn = jax.jit(
            shard_map(
                _body, mesh=self.mesh, in_specs=in_specs, out_specs=out_specs,
                check_rep=False,
            ),
            keep_unused=True,
        )
        self.n_params = n_params

    def put_inputs(self, in_maps):
        jax = self.jax
        per_core = [[np.asarray(m[name]) for name in self.in_names] for m in in_maps]
        concat_in = [
            np.concatenate([per_core[c][i] for c in range(self.n_cores)], axis=0)
            for i in range(self.n_params)
        ]
        self.dev_in = [jax.device_put(a, self.sharding) for a in concat_in]
        self.dev_zeros = [
            jax.device_put(
                np.zeros((self.n_cores * z.shape[0], *z.shape[1:]), z.dtype),
                self.sharding,
            )
            for z in self.zero_outs
        ]
        jax.block_until_ready(self.dev_in)

    def run(self):
        outs = self.fn(*self.dev_in, *self.dev_zeros)
        self.jax.block_until_ready(outs)
        return outs

    def results(self, outs):
        res = []
        for c in range(self.n_cores):
            res.append(
                {
                    name: np.asarray(outs[i]).reshape(
                        self.n_cores, *self.out_avals[i].shape
                    )[c]
                    for i, name in enumerate(self.out_names)
                }
            )
        return res

    def time_runs(self, n=6):
        import time
        ts = []
        for _ in range(n):
            t0 = time.perf_counter()
            self.run()
            ts.append(time.perf_counter() - t0)
        return ts

    def time_pipelined(self, k=32):
        """Amortized per-run time: k executions enqueued back-to-back, one
        blocking wait. Divides out the fixed axon-tunnel await latency that a
        serial wall-clock measurement pays on every run."""
        import time
        jax = self.jax
        t0 = time.perf_counter()
        allouts = [self.fn(*self.dev_in, *self.dev_zeros) for _ in range(k)]
        jax.block_until_ready(allouts)
        return (time.perf_counter() - t0) / k


# ---------------------------------------------------------------------------
# Host-side graph partitioning
# ---------------------------------------------------------------------------


def _host_prep(edge_index):
    src = np.asarray(edge_index[0], dtype=np.int64)
    dst = np.asarray(edge_index[1], dtype=np.int64)
    deg = np.bincount(dst, minlength=N).astype(np.float32) + 1.0
    dinv = (1.0 / np.sqrt(deg)).astype(np.float32)

    core = dst // SR
    dl = dst - core * SR
    tile = dl // P
    dslot = dl % P
    gid = tile // GK
    score = src // SR
    psrc = score * SH + (src - score * SR)
    chunk = psrc // CH
    crel = psrc - chunk * CH

    order = np.lexsort((psrc, tile, chunk, gid, core))
    core_s = core[order]
    tile_s = tile[order]
    g_s = gid[order]
    ch_s = chunk[order]
    crel_s = crel[order]
    dslot_s = dslot[order]

    # per-(c,g,ch) dense segment counts
    segkey = (core_s * NG + g_s) * NCHUNK + ch_s
    seg_cnt = np.bincount(segkey, minlength=NC * NG * NCHUNK).reshape(NC, NG, NCHUNK)
    SEGB = -(-seg_cnt.max(axis=0) // P)                      # [NG, NCHUNK]

    bstart_gc = np.zeros((NG, NCHUNK), np.int64)
    icol_gc = np.zeros((NG, NCHUNK), np.int64)
    TBG = 0
    icol = 0
    for g in range(NG):
        boff = 0
        for ch in range(NCHUNK):
            bstart_gc[g, ch] = boff
            boff += int(SEGB[g, ch])
            icol_gc[g, ch] = icol
            icol += int(SEGB[g, ch]) * P // 16
        TBG = max(TBG, boff)
    ICOL = icol

    # per-edge rank within its (c,g,ch) segment
    segstart = np.zeros(NC * NG * NCHUNK + 1, np.int64)
    np.cumsum(seg_cnt.reshape(-1), out=segstart[1:])
    lin = np.arange(E) - segstart[segkey]

    # per-(c,t,ch) run starts within the segment
    tkey = (core_s * TP + tile_s) * NCHUNK + ch_s
    cnt_tc = np.bincount(tkey, minlength=NC * TP * NCHUNK).reshape(NC, TP, NCHUNK)
    start_tc = np.zeros((NC, TP, NCHUNK), np.int64)
    for g in range(NG):
        ts_ = GROUP_TILES[g]
        run = np.zeros((NC, NCHUNK), np.int64)
        for t in ts_:
            start_tc[:, t, :] = run
            run = run + cnt_tc[:, t, :]

    # cross-core union windows per (t, ch)
    has = cnt_tc > 0
    lo_all = np.where(has, start_tc // P, np.iinfo(np.int64).max)
    hi_all = np.where(has, -(-(start_tc + cnt_tc) // P), 0)
    wlo = lo_all.min(axis=0)            # [TP, NCHUNK]
    whi = hi_all.max(axis=0)
    W_tc = np.maximum(0, whi - wlo)
    wlo = np.where(W_tc > 0, wlo, 0)

    ch_off = np.concatenate([np.zeros((TP, 1), np.int64),
                             np.cumsum(W_tc, axis=1)], axis=1)   # [TP, 5]
    dcol_t = np.zeros((TP,), np.int64)
    dcol = 0
    for t in range(TP):
        dcol_t[t] = dcol
        dcol += int(W_tc[t].sum())
    DCOL = dcol
    WMAX = int(W_tc.sum(axis=1).max())

    import ml_dtypes
    dsel_tab = np.full((NC, P, DCOL), -1.0, ml_dtypes.bfloat16)
    col = dcol_t[tile_s] + ch_off[tile_s, ch_s] + (lin // P - wlo[tile_s, ch_s])
    dsel_tab[core_s, lin % P, col] = dslot_s.astype(ml_dtypes.bfloat16)

    idx_tab = np.full((NC, P, ICOL), -1, np.int16)
    icolumn = icol_gc[g_s, ch_s] + lin // 16
    irow = (lin % 16).astype(np.int64)
    for k in range(8):
        idx_tab[core_s, 16 * k + irow, icolumn] = crel_s.astype(np.int16)

    cnts = seg_cnt.reshape(NC, NG * NCHUNK).astype(np.int32)

    meta = {
        "SEGB": SEGB, "TBG": TBG, "ICOL": ICOL, "DCOL": DCOL, "WMAX": WMAX,
        "bstart_gc": bstart_gc, "icol_gc": icol_gc,
        "wlo": wlo, "W_tc": W_tc, "ch_off": ch_off, "dcol_t": dcol_t,
    }
    return dinv, idx_tab, dsel_tab, cnts, meta


def _fold_weights(inputs):
    """Fold BN into W/b; pad layer-4 to 128 outputs and W5 to 128 inputs."""
    Ws, bs = [], []
    for i in range(1, 6):
        W = np.asarray(inputs[f"W{i}"], np.float32)
        b = np.asarray(inputs[f"b{i}"], np.float32)
        if i <= 4:
            g = np.asarray(inputs[f"g{i}"], np.float32)
            be = np.asarray(inputs[f"be{i}"], np.float32)
            rm = np.asarray(inputs[f"rm{i}"], np.float32)
            rv = np.asarray(inputs[f"rv{i}"], np.float32)
            s = g / np.sqrt(rv + EPS)
            W = W * s[None, :]
            b = b * s + be - rm * s
        Ws.append(W)
        bs.append(b)
    W4 = np.zeros((H, H), np.float32)
    W4[:, :H // 2] = Ws[3]
    W5 = np.zeros((H, C), np.float32)
    W5[:H // 2] = Ws[4]
    WA = [Ws[0], Ws[1], Ws[2], W4, np.eye(H, dtype=np.float32)]
    import ml_dtypes
    brow = np.zeros((4, P), ml_dtypes.bfloat16)
    for l in range(4):
        brow[l, :len(bs[l])] = bs[l].astype(ml_dtypes.bfloat16)
    return WA, W5, brow, bs[4]


# ---------------------------------------------------------------------------
# Device program
# ---------------------------------------------------------------------------


def _build_nc(meta):
    NLAY = int(os.environ.get("GCN_LAYERS", 5))
    import concourse.bass as bass
    import concourse.mybir as mybir
    from concourse.tile import TileContext
    from concourse import library_config

    _apply_tile_patch()

    SEGB = meta["SEGB"]
    TBG, ICOL, DCOL, WMAX = meta["TBG"], meta["ICOL"], meta["DCOL"], meta["WMAX"]
    bstart_gc, icol_gc = meta["bstart_gc"], meta["icol_gc"]
    wlo, W_tc, ch_off, dcol_t = meta["wlo"], meta["W_tc"], meta["ch_off"], meta["dcol_t"]

    f32 = mybir.dt.float32
    bf16 = mybir.dt.bfloat16
    i16 = mybir.dt.int16
    i32 = mybir.dt.int32
    nc = bass.Bass("TRN2", target_bir_lowering=False, debug=False, num_swdge_queues=4)

    xT_in = nc.declare_dram_parameter("xT", [IN, SH], f32, isOutput=False)
    dinv_in = nc.declare_dram_parameter("dinv", [P, TP], f32, isOutput=False)
    dinv2_in = nc.declare_dram_parameter("dinv2", [P, TP], f32, isOutput=False)
    sqd_in = nc.declare_dram_parameter("sqd", [1, SH], bf16, isOutput=False)
    idx_in = nc.declare_dram_parameter("idx", [P, ICOL], i16, isOutput=False)
    dsel_in = nc.declare_dram_parameter("dsel", [P, DCOL], bf16, isOutput=False)
    cnts_in = nc.declare_dram_parameter("cnts", [1, NG * NCHUNK], i32, isOutput=False)
    WA_in = [nc.declare_dram_parameter(f"WA{l}", [H, H], f32, isOutput=False) for l in range(5)]
    W5_in = nc.declare_dram_parameter("W5", [H, C], f32, isOutput=False)
    brow_in = nc.declare_dram_parameter("brow", [4, P], bf16, isOutput=False)
    iota_in = nc.declare_dram_parameter("iota", [P, P], f32, isOutput=False)
    ident_in = nc.declare_dram_parameter("ident", [P, P], f32, isOutput=False)
    y_out = nc.declare_dram_parameter("y", [C, SH], f32, isOutput=True)

    in_b = [nc.dram_tensor(f"in_b{l}", [SH, H], bf16) for l in range(5)]
    hs_full = [
        nc.dram_tensor(f"hs_full{l}", [NF, H], bf16, addr_space="Shared")
        for l in range(5)
    ]

    with TileContext(nc) as tc:
        with (
            tc.tile_pool(name="const", bufs=1) as cpool,
            tc.tile_pool(name="gath", bufs=2) as gpool,
            tc.tile_pool(name="sall", bufs=3) as spool,
            tc.tile_pool(name="work", bufs=4) as wpool,
            tc.tile_pool(name="ps_h", bufs=2, space="PSUM") as ps_h,
            tc.tile_pool(name="ps_a", bufs=4, space="PSUM") as ps_a,
            tc.tile_pool(name="ps_y", bufs=2, space="PSUM") as ps_y,
        ):
            nc.gpsimd.load_library(library_config.mlp)
            nreg = nc.alloc_register(mybir.EngineType.Pool, "nidx")

            WAt = []
            for l in range(5):
                w = cpool.tile([H, H], f32, name=f"WA{l}")
                nc.sync.dma_start(out=w[:], in_=WA_in[l][:])
                WAt.append(w)
            W5t = cpool.tile([H, C], f32, name="W5t")
            nc.sync.dma_start(out=W5t[:], in_=W5_in[:])
            brow_t = []
            for l in range(4):
                b = cpool.tile([1, P], bf16, name=f"brow{l}")
                nc.sync.dma_start(out=b[:], in_=brow_in.ap()[l:l + 1, :])
                brow_t.append(b)
            iota_t = cpool.tile([P, P], f32)
            nc.sync.dma_start(out=iota_t[:], in_=iota_in[:])
            iota_w = cpool.tile([P, WMAX, P], bf16, name="iota_w")
            for w in range(WMAX):
                nc.vector.tensor_copy(out=iota_w[:, w, :], in_=iota_t[:])
            ident_t = cpool.tile([P, P], f32)
            nc.sync.dma_start(out=ident_t[:], in_=ident_in[:])
            ident_b = cpool.tile([P, P], bf16)
            nc.vector.tensor_copy(out=ident_b[:], in_=ident_t[:])
            dinv_t = cpool.tile([P, TP], f32)
            nc.sync.dma_start(out=dinv_t[:], in_=dinv_in[:])
            dinv2_t = cpool.tile([P, TP], f32)
            nc.sync.dma_start(out=dinv2_t[:], in_=dinv2_in[:])
            sqd_t = cpool.tile([1, SH], bf16, name="sqd")
            nc.sync.dma_start(out=sqd_t[:], in_=sqd_in[:])
            cnts_t = cpool.tile([1, NG * NCHUNK], i32, name="cnts")
            nc.sync.dma_start(out=cnts_t[:], in_=cnts_in[:])
            idx_sb = cpool.tile([P, ICOL], i16, name="idx_sb")
            nc.scalar.dma_start(out=idx_sb[:], in_=idx_in[:])
            dsel_sb = cpool.tile([P, DCOL], bf16, name="dsel_sb")
            nc.scalar.dma_start(out=dsel_sb[:], in_=dsel_in[:])
            actT = cpool.tile([P, SH], f32, name="actT")
            nc.sync.dma_start(out=actT[:IN, :], in_=xT_in[:])
            hs_keep = cpool.tile([P, TP * H], bf16, name="hs_keep")

            def phase_a(l, t):
                ps = ps_h.tile([P, H], f32, tag="ps_h")
                nc.tensor.matmul(
                    out=ps[:], lhsT=actT[:, t * P:(t + 1) * P], rhs=WAt[l][:],
                    start=True, stop=True,
                )
                sc = dinv_t if l == 0 else dinv2_t
                nc.vector.tensor_scalar_mul(
                    out=hs_keep[:, t * H:(t + 1) * H], in0=ps[:],
                    scalar1=sc[:, t:t + 1],
                )
                nc.sync.dma_start(
                    out=in_b[l].ap()[t * P:(t + 1) * P, :],
                    in_=hs_keep[:, t * H:(t + 1) * H],
                )

            def allgather(l):
                nc.gpsimd.collective_compute(
                    "AllGather",
                    mybir.AluOpType.bypass,
                    ins=[in_b[l][:]],
                    outs=[hs_full[l][:]],
                    replica_groups=[list(range(NC))],
                )

            for t in range(TP):
                phase_a(0, t)
            allgather(0)

            for l in range(NLAY):
                for g in range(NG):
                    gt = gpool.tile([P, TBG, H], bf16, tag="gt")
                    if l == 0 and g < 2:
                        # zero the two gather buffers once: never-gathered tail
                        # blocks (short per-core segments) must not hold NaN bits
                        nc.vector.memset(gt[:], 0.0)
                    for ch in range(NCHUNK):
                        nb = int(SEGB[g, ch])
                        if nb == 0:
                            continue
                        n = nb * P
                        k = g * NCHUNK + ch
                        nc.gpsimd.reg_load(nreg, cnts_t[0:1, k:k + 1])
                        bs = int(bstart_gc[g, ch])
                        nc.gpsimd.dma_gather(
                            out_ap=gt[:, bs:bs + nb, :],
                            in_ap=hs_full[l].ap()[CH_BASE[ch]:CH_BASE[ch] + CH_SIZE[ch], :],
                            idxs_ap=idx_sb[:, int(icol_gc[g, ch]):int(icol_gc[g, ch]) + n // 16],
                            num_idxs=n,
                            num_idxs_reg=nreg,
                            elem_size=H,
                            single_packet=False,
                            queue_num=(g + ch) % 4,
                        )
                    for t in GROUP_TILES[g]:
                        wt = int(W_tc[t].sum())
                        pa = ps_a.tile([P, P], f32, tag="pa")
                        if wt > 0:
                            S_all = spool.tile([P, WMAX, P], bf16, tag="S")
                            d0 = int(dcol_t[t])
                            nc.vector.tensor_tensor(
                                out=S_all[:, :wt, :],
                                in0=iota_w[:, :wt, :],
                                in1=dsel_sb[:, d0:d0 + wt].unsqueeze(2).broadcast_to([P, wt, P]),
                                op=mybir.AluOpType.is_equal,
                            )
                        first = True
                        for ch in range(NCHUNK):
                            base = int(bstart_gc[g, ch] + wlo[t, ch])
                            sbase = int(ch_off[t, ch])
                            for w in range(int(W_tc[t, ch])):
                                nc.tensor.matmul(
                                    out=pa[:],
                                    lhsT=gt[:, base + w, :],
                                    rhs=S_all[:, sbase + w, :],
                                    start=first, stop=False,
                                )
                                first = False
                        nc.tensor.matmul(
                            out=pa[:], lhsT=hs_keep[:, t * H:(t + 1) * H],
                            rhs=ident_b[:], start=first, stop=(l == 4),
                        )
                        if l < 4:
                            nc.tensor.matmul(
                                out=pa[:], lhsT=brow_t[l][:],
                                rhs=sqd_t[0:1, t * P:(t + 1) * P],
                                start=False, stop=True,
                            )
                            nc.scalar.activation(
                                out=actT[:, t * P:(t + 1) * P], in_=pa[:],
                                func=mybir.ActivationFunctionType.Relu,
                            )
                            phase_a(l + 1, t)
                        else:
                            c5 = wpool.tile([P, P], f32, tag="c5")
                            nc.vector.tensor_copy(out=c5[:], in_=pa[:])
                            po = ps_y.tile([C, P], f32, tag="po")
                            nc.tensor.matmul(out=po[:], lhsT=W5t[:], rhs=c5[:],
                                             start=True, stop=True)
                            yt = wpool.tile([C, P], f32, tag="yt")
                            nc.vector.tensor_copy(out=yt[:], in_=po[:])
                            nc.sync.dma_start(
                                out=y_out.ap()[:, t * P:(t + 1) * P], in_=yt[:]
                            )
                if l < 4:
                    allgather(l + 1)
    return nc


def kernel(**inputs):
    import ml_dtypes
    edge_index = np.asarray(inputs["edge_index"])
    key = edge_index.tobytes()[:64]
    if "prep" not in _cache or _cache.get("key") != key:
        _cache["key"] = key
        _cache["prep"] = _host_prep(edge_index)
        _cache.pop("runner", None)
    dinv, idx_tab, dsel_tab, cnts, meta = _cache["prep"]
    WA, W5, brow, b5 = _fold_weights(inputs)

    x = np.asarray(inputs["x"], np.float32)
    xpad = np.zeros((NC, SH, IN), np.float32)
    xpad[:, :SR] = x.reshape(NC, SR, IN)
    dinvpad = np.ones((NC, SH), np.float32)
    dinvpad[:, :SR] = dinv.reshape(NC, SR)

    iota = np.tile(np.arange(P, dtype=np.float32)[None, :], (P, 1))

    if "runner" not in _cache:
        nc = _build_nc(meta)
        _cache["runner"] = _SpmdRunner(nc, NC)
    r = _cache["runner"]

    in_maps = []
    for c in range(NC):
        dvc = dinvpad[c].reshape(TP, P)
        m = {
            "xT": np.ascontiguousarray(xpad[c].T),
            "dinv": np.ascontiguousarray(dvc.T),
            "dinv2": np.ascontiguousarray((dvc * dvc).T),
            "sqd": (1.0 / dinvpad[c])[None, :].astype(ml_dtypes.bfloat16),
            "idx": idx_tab[c],
            "dsel": dsel_tab[c],
            "cnts": cnts[c][None, :],
            "iota": iota,
            "ident": np.eye(P, dtype=np.float32),
            "W5": W5,
            "brow": brow,
        }
        for l in range(5):
            m[f"WA{l}"] = WA[l]
        in_maps.append(m)

    r.put_inputs(in_maps)
    outs = r.run()
    res = r.results(outs)
    ys = []
    for c in range(NC):
        yT = res[c]["y"].T.astype(np.float32)          # [SH, 2] raw W5^T pa
        ys.append(dinvpad[c][:SR, None] * yT[:SR] + b5[None, :])
    y = np.concatenate(ys, axis=0)[:N]
    return np.ascontiguousarray(y, dtype=np.float32)
